# revision 1
# baseline (speedup 1.0000x reference)
"""Trainium2 Bass kernel for nn_AttentionBlock (GroupNorm + 1x1-conv QKV
self-attention + 1x1-conv out-proj + residual).

Full input shapes: x (8, 256, 64, 64) f32, gn_weight/gn_bias (256,),
qkv_w (768, 256), qkv_b (768,), out_w (256, 256), out_b (256,).

Sharding: data-parallel over batch — one batch item per NeuronCore (8 cores).

Per-core layout: channels on partitions, pixels on the free dim.
  xn (c, hw) -> kT = Wk@xn in (c, j) layout (scores lhsT), V = xn^T@Wv^T
  directly in (j, c) layout (PV lhsT), so no PE transposes are needed.
  Scores are computed transposed, S^T (j, i), softmax runs without max
  subtraction (scores ~ N(0,1) here; exp overflow needs |s| > 88), the
  denominator is a DVE tree reduction + ones-matmul partition reduction, and
  P^T @ V accumulates in PSUM over j producing attn-out directly in (c, i)
  layout for the out-projection.

Precision split: the score path (xn, kt, qt, qkv weights) runs in float32r
(TF32, 1 PE cycle/row); the post-softmax path (exp(S^T), V, attn, out_w)
runs in bf16 — softmax weights are normalized by the sum of the same bf16
values, so the quantization largely cancels. PSUM accumulation is fp32.

The per-block tail (denominator finish, normalize, out-proj, residual) for
block ib-1 is emitted between scores(ib) and PV(ib) so the PE never waits
on the DVE/ACT chain.

Host-side folds: q weights/bias pre-scaled by 1/sqrt(c); v bias folded into
the out-proj bias (rows of softmax sum to 1 -> attn@(V + 1 vb^T) =
attn@V + vb, so obias = out_w @ vb + out_b).
"""

import ml_dtypes
import numpy as np

import concourse.bass as bass
import concourse.tile as tile
from concourse import bacc, mybir
from concourse.bass_utils import run_bass_kernel_spmd

F32 = mybir.dt.float32
F32R = mybir.dt.float32r
BF16 = mybir.dt.bfloat16
AF = mybir.ActivationFunctionType
OP = mybir.AluOpType

B = 8          # batch (= cores)
C = 256        # channels
P = 128        # partitions
NCC = C // P   # channel chunks (2)
G = 32         # groups
GS = C // G    # channels per group (8)
GPC = P // GS  # groups per partition chunk (16)
EPS = 1e-5


def build(hw=4096, iblk=512):
    """Build the per-core Bass program. hw = pixels per image (4096 full)."""
    assert hw % 512 == 0 and hw % iblk == 0 and iblk >= 256
    njt = hw // P      # j tiles of 128 (32 full size)
    nib = hw // iblk   # i blocks (8 full size)
    njb = hw // 512    # 512-wide chunks for the k conv

    nc = bacc.Bacc("TRN2", target_bir_lowering=False, debug=False, num_devices=B)

    nxc = hw // 512
    x_d = nc.dram_tensor("x", [NCC, P, hw], F32, kind="ExternalInput").ap()
    qkv_wt_d = nc.dram_tensor(
        "qkv_wt", [NCC, P, 3 * C], F32, kind="ExternalInput"
    ).ap()
    out_wt_d = nc.dram_tensor(
        "out_wt", [NCC, P, C], BF16, kind="ExternalInput"
    ).ap()
    qkv_b4_d = nc.dram_tensor("qkv_b4", [P, 4], F32, kind="ExternalInput").ap()
    obias_d = nc.dram_tensor("obias", [P, NCC], F32, kind="ExternalInput").ap()
    gn_w_d = nc.dram_tensor("gn_w", [P, NCC], F32, kind="ExternalInput").ap()
    gn_b_d = nc.dram_tensor("gn_b", [P, NCC], F32, kind="ExternalInput").ap()
    gmask_d = nc.dram_tensor("gmask", [P, GPC], F32, kind="ExternalInput").ap()
    gmaskT_d = nc.dram_tensor("gmaskT", [GPC, P], F32, kind="ExternalInput").ap()
    onesc_d = nc.dram_tensor("onesc", [P, 1], F32, kind="ExternalInput").ap()
    y_d = nc.dram_tensor("y", [NCC, P, hw], F32, kind="ExternalOutput").ap()

    with tile.TileContext(nc) as tc:
        with (
            tc.tile_pool(name="const", bufs=1) as cst,
            tc.tile_pool(name="kt", bufs=1) as ktp,
            tc.tile_pool(name="v", bufs=1) as vp,
            tc.tile_pool(name="xn", bufs=1) as xnp,
            tc.tile_pool(name="es", bufs=1) as esp,
            tc.tile_pool(name="work", bufs=2) as wp,
            tc.tile_pool(name="stat", bufs=2) as sp,
            tc.tile_pool(name="ps_s", bufs=2, space="PSUM") as ps_s,
            tc.tile_pool(name="ps_pv", bufs=4, space="PSUM") as ps_pv,
            tc.tile_pool(name="ps_m", bufs=2, space="PSUM") as ps_m,
        ):
            # ---- constants / weights to SBUF ----
            qkv_wt = cst.tile([P, NCC, 3 * C], F32R)
            out_wt = cst.tile([P, NCC, C], BF16)
            qkv_b4 = cst.tile([P, 4], F32)
            obias = cst.tile([P, NCC], F32)
            gn_w = cst.tile([P, NCC], F32)
            gn_b = cst.tile([P, NCC], F32)
            gmask = cst.tile([P, GPC], F32)
            gmaskT = cst.tile([GPC, P], F32)
            onesR = cst.tile([P, 1], F32R)    # fp32r ones column (denominator)
            ones1 = cst.tile([1, P], F32)     # fp32 ones row (broadcast matmul)
            eps_t = cst.tile([P, 1], F32)
            for cc in range(NCC):
                nc.sync.dma_start(
                    out=qkv_wt[:, cc, :], in_=qkv_wt_d[cc].bitcast(F32R)
                )
                nc.sync.dma_start(out=out_wt[:, cc, :], in_=out_wt_d[cc])
            nc.sync.dma_start(out=qkv_b4, in_=qkv_b4_d[:, :])
            nc.sync.dma_start(out=obias, in_=obias_d[:, :])
            nc.sync.dma_start(out=gn_w, in_=gn_w_d[:, :])
            nc.sync.dma_start(out=gn_b, in_=gn_b_d[:, :])
            nc.sync.dma_start(out=gmask, in_=gmask_d[:, :])
            nc.sync.dma_start(out=gmaskT, in_=gmaskT_d[:, :])
            nc.sync.dma_start(out=onesR, in_=onesc_d[:, :].bitcast(F32R))
            nc.vector.memset(ones1, 1.0)
            nc.vector.memset(eps_t, EPS)

            # big persistent tensors
            kt = ktp.tile([P, NCC, hw], F32R)          # k in (c, j) layout
            v_sb = vp.tile([P, njt, C], BF16)          # v in (j, c) layout
            xn = xnp.tile([P, NCC, hw], F32R)          # normalized x

            # x staged into the region later reused for exp(S^T); chunked and
            # contiguous in DRAM so bn_stats can chase the DMA
            xs = esp.tile([P, NCC, hw], F32, tag="es")
            for cc in range(NCC):
                for h2 in range(nxc):
                    nc.sync.dma_start(
                        out=xs[:, cc, h2 * 512:(h2 + 1) * 512],
                        in_=x_d[cc, :, h2 * 512:(h2 + 1) * 512],
                    )

            # ---- GroupNorm ----
            nsg = hw // 512
            for cc in range(NCC):
                stats = sp.tile([P, nsg, 6], F32, tag="bnst")
                for sg in range(nsg):
                    nc.vector.bn_stats(
                        out=stats[:, sg, :], in_=xs[:, cc, sg * 512:(sg + 1) * 512]
                    )
                mv = sp.tile([P, 2], F32, tag="mv")
                nc.vector.bn_aggr(out=mv, in_=stats)
                # t = [mean, E[x^2]] per row
                t = sp.tile([P, 2], F32, tag="t2")
                nc.vector.tensor_copy(t[:, 0:1], mv[:, 0:1])
                nc.vector.tensor_mul(t[:, 1:2], mv[:, 0:1], mv[:, 0:1])
                nc.vector.tensor_add(t[:, 1:2], t[:, 1:2], mv[:, 1:2])
                # sum over the 8 rows of each group (fp32 matmul, N=2)
                gsum = ps_m.tile([GPC, 2], F32, tag="mm")
                nc.tensor.matmul(gsum, gmask, t, start=True, stop=True)
                gstat = sp.tile([GPC, 2], F32, tag="gstat")
                nc.scalar.activation(gstat, gsum, AF.Copy, scale=1.0 / GS)
                gvar = sp.tile([GPC, 1], F32, tag="gvar")
                nc.vector.tensor_mul(gvar, gstat[:, 0:1], gstat[:, 0:1])
                nc.vector.tensor_sub(gvar, gstat[:, 1:2], gvar)
                nc.scalar.activation(gvar, gvar, AF.Sqrt, bias=eps_t[0:GPC, :])
                nc.vector.reciprocal(gvar, gvar)       # rstd per group
                gmr = sp.tile([GPC, 2], F32, tag="gmr")
                nc.vector.tensor_copy(gmr[:, 0:1], gstat[:, 0:1])
                nc.vector.tensor_copy(gmr[:, 1:2], gvar)
                # broadcast group stats back to the 128 rows
                bc = ps_m.tile([P, 2], F32, tag="mm")
                nc.tensor.matmul(bc, gmaskT, gmr, start=True, stop=True)
                rowst = sp.tile([P, 2], F32, tag="rowst")
                nc.vector.tensor_copy(rowst, bc)
                # xn = x * (rstd*w) + (b - mean*rstd*w)
                a_t = sp.tile([P, 1], F32, tag="a")
                b_t = sp.tile([P, 1], F32, tag="b")
                nc.vector.tensor_mul(a_t, rowst[:, 1:2], gn_w[:, cc:cc + 1])
                nc.vector.tensor_mul(b_t, rowst[:, 0:1], a_t)
                nc.vector.tensor_sub(b_t, gn_b[:, cc:cc + 1], b_t)
                nc.vector.tensor_scalar(
                    out=xn[:, cc, :], in0=xs[:, cc, :],
                    scalar1=a_t, scalar2=b_t, op0=OP.mult, op1=OP.add,
                )

            # ---- k conv: kT[c_out, j] (+ bias) ----
            for oc in range(NCC):
                for jb in range(njb):
                    pk = ps_s.tile([P, 512], F32, tag="mm")
                    for cc in range(NCC):
                        nc.tensor.matmul(
                            pk,
                            qkv_wt[:, cc, C + oc * P:C + (oc + 1) * P],
                            xn[:, cc, jb * 512:(jb + 1) * 512],
                            start=(cc == 0), stop=(cc == NCC - 1),
                        )
                    nc.scalar.activation(
                        kt[:, oc, jb * 512:(jb + 1) * 512], pk, AF.Identity,
                        bias=qkv_b4[:, 2 + oc:3 + oc],
                    )

            # ---- attention: software-pipelined across i-blocks.
            # The softmax denominator is accumulated in 4 partial-sum chains
            # that chase the exp stream on the DVE, so the partition-reduce
            # matmul (dfull) is ready right after PV. The 1/denom broadcast
            # goes over an idle DMA queue; residual adds go to GpSimd. ----
            st = {}

            def emit_qt(ib):
                isl = slice(ib * iblk, (ib + 1) * iblk)
                qt = wp.tile([P, NCC, iblk], F32R, tag="qt", name=f"qt{ib}")
                for oc in range(NCC):
                    pq = ps_m.tile([P, iblk], F32, tag="mm", name=f"pq{ib}_{oc}")
                    for cc in range(NCC):
                        nc.tensor.matmul(
                            pq,
                            qkv_wt[:, cc, oc * P:(oc + 1) * P],
                            xn[:, cc, isl],
                            start=(cc == 0), stop=(cc == NCC - 1),
                        )
                    nc.vector.tensor_scalar(
                        out=qt[:, oc, :], in0=pq, scalar1=qkv_b4[:, oc:oc + 1],
                        scalar2=None, op0=OP.add,
                    )
                st[ib] = {"qt": qt}

            def emit_scores(ib):
                # scores + exp + incremental denominator chains (4 x 8 jt)
                es = esp.tile([P, njt, iblk], BF16, tag="es", name=f"es{ib}")
                pc = wp.tile([P, 4, iblk], F32, tag="pc", name=f"pc{ib}")
                qt = st[ib]["qt"]
                span = njt // 4
                for jt in range(njt):
                    ps = ps_s.tile([P, iblk], F32, tag="mm", name=f"ps{ib}_{jt}")
                    for cc in range(NCC):
                        nc.tensor.matmul(
                            ps,
                            kt[:, cc, jt * P:(jt + 1) * P],
                            qt[:, cc, :],
                            start=(cc == 0), stop=(cc == NCC - 1),
                        )
                    nc.scalar.activation(es[:, jt, :], ps, AF.Exp)
                    k, r = divmod(jt, span)
                    if r == 1:
                        nc.vector.tensor_add(
                            pc[:, k, :], es[:, jt - 1, :], es[:, jt, :]
                        )
                    elif r > 1:
                        nc.vector.tensor_add(
                            pc[:, k, :], pc[:, k, :], es[:, jt, :]
                        )
                # combine chains; final sum rounded to f32r for the matmul
                nc.vector.tensor_add(pc[:, 0, :], pc[:, 0, :], pc[:, 1, :])
                nc.vector.tensor_add(pc[:, 2, :], pc[:, 2, :], pc[:, 3, :])
                acc = wp.tile([P, iblk], F32R, tag="acc", name=f"acc{ib}")
                nc.vector.tensor_add(acc, pc[:, 0, :], pc[:, 2, :])
                st[ib]["es"] = es
                st[ib]["acc"] = acc

            def emit_pv(ib):
                es = st[ib]["es"]
                pvp = [
                    ps_pv.tile([P, iblk], F32, tag="pv", name=f"pv{ib}_{oc}")
                    for oc in range(NCC)
                ]
                for oc in range(NCC):
                    for jt in range(njt):
                        nc.tensor.matmul(
                            pvp[oc],
                            v_sb[:, jt, oc * P:(oc + 1) * P],
                            es[:, jt, :],
                            start=(jt == 0), stop=(jt == njt - 1),
                        )
                st[ib]["pvp"] = pvp

            def emit_denfinish(ib):
                # partition-reduce, fast reciprocal, broadcast via DMA
                dfull = ps_m.tile([P, iblk], F32, tag="mm", name=f"dful{ib}")
                nc.tensor.matmul(
                    dfull[0:1, :], onesR, st[ib]["acc"], start=True, stop=True
                )
                rd = wp.tile([1, iblk], F32, tag="rd", name=f"rd{ib}")
                nc.vector.reciprocal_approx_fast(rd, dfull[0:1, :])
                st[ib]["rd"] = rd

            def emit_normalize(ib):
                rbp = ps_m.tile([P, iblk], F32, tag="mm", name=f"rbp{ib}")
                nc.tensor.matmul(rbp, ones1, st[ib]["rd"], start=True, stop=True)
                rb = wp.tile([P, iblk], F32, tag="rb", name=f"rb{ib}")
                nc.vector.tensor_copy(rb, rbp)
                attn = wp.tile([P, NCC, iblk], BF16, tag="attn", name=f"at{ib}")
                for oc in range(NCC):
                    nc.vector.tensor_mul(attn[:, oc, :], st[ib]["pvp"][oc], rb)
                st[ib]["attn"] = attn

            def emit_outproj(ib):
                attn = st[ib]["attn"]
                isl = slice(ib * iblk, (ib + 1) * iblk)
                xres = wp.tile([P, NCC, iblk], F32, tag="xres", name=f"xr{ib}")
                for cc in range(NCC):
                    nc.sync.dma_start(out=xres[:, cc, :], in_=x_d[cc, :, isl])
                for o2 in range(NCC):
                    py = ps_m.tile([P, iblk], F32, tag="mm", name=f"py{ib}_{o2}")
                    for cc in range(NCC):
                        nc.tensor.matmul(
                            py,
                            out_wt[:, cc, o2 * P:(o2 + 1) * P],
                            attn[:, cc, :],
                            start=(cc == 0), stop=(cc == NCC - 1),
                        )
                    ytmp = wp.tile([P, iblk], F32, tag="ytmp", name=f"yt{ib}_{o2}")
                    nc.scalar.activation(
                        ytmp, py, AF.Identity, bias=obias[:, o2:o2 + 1]
                    )
                    yo = wp.tile([P, iblk], F32, tag="yo", name=f"yo{ib}_{o2}")
                    nc.vector.tensor_add(yo, ytmp, xres[:, o2, :])
                    nc.sync.dma_start(out=y_d[o2, :, isl], in_=yo)
                del st[ib]

            emit_qt(0)
            emit_scores(0)

            # ---- v conv, directly in (j, c) layout; bias folded into obias ----
            for jt in range(njt):
                pv = ps_s.tile([P, C], F32, tag="mm")
                for cc in range(NCC):
                    nc.tensor.matmul(
                        pv,
                        xn[:, cc, jt * P:(jt + 1) * P],
                        qkv_wt[:, cc, 2 * C:3 * C],
                        start=(cc == 0), stop=(cc == NCC - 1),
                    )
                nc.scalar.activation(v_sb[:, jt, :], pv, AF.Copy)

            emit_pv(0)
            emit_denfinish(0)
            for ib in range(1, nib):
                emit_qt(ib)
                emit_normalize(ib - 1)
                emit_scores(ib)
                emit_outproj(ib - 1)
                emit_pv(ib)
                emit_denfinish(ib)
            emit_normalize(nib - 1)
            emit_outproj(nib - 1)

    nc.compile()
    return nc


def prep_inputs(x, gn_weight, gn_bias, qkv_w, qkv_b, out_w, out_b, hw=4096):
    """Host-side layout prep. Returns per-core input maps."""
    b = x.shape[0]
    scale = 1.0 / np.sqrt(np.float32(C))
    wq = qkv_w[:C] * scale
    qkv_wt = np.ascontiguousarray(
        np.concatenate([wq, qkv_w[C:]], axis=0).T.reshape(NCC, P, 3 * C)
    ).astype(np.float32)
    out_wt = np.ascontiguousarray(out_w.T.reshape(NCC, P, C)).astype(
        ml_dtypes.bfloat16
    )
    qb = qkv_b[:C] * scale
    kb = qkv_b[C:2 * C]
    vb = qkv_b[2 * C:]
    qkv_b4 = np.ascontiguousarray(
        np.stack([qb[:P], qb[P:], kb[:P], kb[P:]], axis=1)
    ).astype(np.float32)
    ob = out_b + out_w @ vb
    obias = np.ascontiguousarray(ob.reshape(NCC, P).T).astype(np.float32)
    gn_w2 = np.ascontiguousarray(gn_weight.reshape(NCC, P).T).astype(np.float32)
    gn_b2 = np.ascontiguousarray(gn_bias.reshape(NCC, P).T).astype(np.float32)
    gmask = np.zeros((P, GPC), np.float32)
    gmask[np.arange(P), np.arange(P) // GS] = 1.0
    gmaskT = np.ascontiguousarray(gmask.T)

    shared = dict(
        qkv_wt=qkv_wt, out_wt=out_wt, qkv_b4=qkv_b4, obias=obias,
        gn_w=gn_w2, gn_b=gn_b2, gmask=gmask, gmaskT=gmaskT,
        onesc=np.ones((P, 1), np.float32),
    )
    in_maps = []
    for i in range(b):
        m = dict(shared)
        m["x"] = np.ascontiguousarray(
            x[i].reshape(C, hw).reshape(NCC, P, hw)
        ).astype(np.float32)
        in_maps.append(m)
    return in_maps


_NC_CACHE = {}


def get_nc(hw=4096, iblk=512):
    key = (hw, iblk)
    if key not in _NC_CACHE:
        _NC_CACHE[key] = build(hw, iblk)
    return _NC_CACHE[key]


def kernel(x, gn_weight, gn_bias, qkv_w, qkv_b, out_w, out_b):
    b, c, h, w = x.shape
    assert (b, c) == (B, C)
    hw = h * w
    nc = get_nc(hw=hw)
    in_maps = prep_inputs(x, gn_weight, gn_bias, qkv_w, qkv_b, out_w, out_b, hw=hw)
    res = run_bass_kernel_spmd(nc, in_maps, core_ids=list(range(B)))
    out = np.stack(
        [res.results[i]["y"].reshape(C, h, w) for i in range(b)]
    ).astype(np.float32)
    return out

